# revision 14
# baseline (speedup 1.0000x reference)
"""CVRAE next-action Trainium2 Bass kernel.

Sharding: data-parallel over batch. B=256 -> 8 cores x 32. The scan is
sequential in time but embarrassingly parallel over batch; the only
cross-batch quantity is the scalar KLD, accumulated per-core and summed
on host.

Math decomposition (validated vs jax reference at 5e-7 rel in fp32):
  px = relu(one_hot(x) @ phi_x_w.T + b) takes only A=64 values, so its
  contributions through enc_w1 / gru_wih are folded into per-symbol
  tables M1=[A,H], M2=[A,3H], applied per step via a one-hot matmul that
  also injects biases through an appended ones-row.

Per-core per-step matmuls (activation-stationary: the small [K,32]
batch tile is the PE stationary operand, weight matrices stream as the
moving operand at 1 column/cycle in bf16; fp32 PSUM accumulation):
  enc1 = M1[x] + h@W1b.T               pri1 = h@pri_w1.T
  enc2 = eh@enc_w2.T                   pri2 = ph@pri_w2.T
  pz   = relu(z@phi_z_w.T)             rz/inn = M2[x] + pz@Wih_b.T
  rz  += h@Whh_rz.T                    hn   = h@Whh_n.T
Feature-major stationaries are produced by PE transposes (packed 4 per
PSUM bank, drained by ScalarE).

TRN2 has no Softplus/Ln/Sigmoid+Tanh activation tables, so everything
uses the single Tanh table: sigmoid(x)=(1+tanh(x/2))/2 (exact),
1+e^v = 2/(1-tanh(v/2)) (exact), and ln(y) is a pure-ALU bitfield log:
exponent via magic-number float bias, deg-5 minimax poly on the
mantissa (max err ~1e-5).
"""

import sys
from contextlib import ExitStack

import numpy as np

if "/opt/trn_rl_repo" not in sys.path:
    sys.path.insert(0, "/opt/trn_rl_repo")

import ml_dtypes

B, S, A, H, Z = 256, 512, 64, 1024, 256
H3 = 3 * H
NCORES = 8
BL = B // NCORES  # 32
UNROLL = 4

BF16 = ml_dtypes.bfloat16


def _host_prep(inputs):
    """Fold/transpose weights on host; build per-core input maps."""
    f32 = np.float32
    g = {k: np.asarray(v) for k, v in inputs.items()}
    x = g["x"].astype(np.int64)  # [B, S]
    h0 = g["h0"].astype(f32)
    eps = g["eps"].astype(f32)  # [S+1, B, Z]

    W = lambda k: g[k].astype(f32)

    relu_table = np.maximum(W("phi_x_w").T + W("phi_x_b")[None, :], 0.0)  # [A,H]
    M1 = relu_table @ W("enc_w1")[:, :H].T  # [A, H]
    M2 = relu_table @ W("gru_wih")[:, :H].T  # [A, 3H]
    bias_ihhh = W("gru_bih") + W("gru_bhh")

    # M12E [A+1, H + 3H]: cols [enc1 | gru_rz | gru_inn]; last row = biases
    m12e = np.zeros((A + 1, H + H3), f32)
    m12e[:A, :H] = M1
    m12e[:A, H : 3 * H] = M2[:, : 2 * H]
    m12e[:A, 3 * H :] = M2[:, 2 * H :]
    m12e[A, :H] = W("enc_b1")
    m12e[A, H : 3 * H] = bias_ihhh[: 2 * H]
    m12e[A, 3 * H :] = W("gru_bih")[2 * H :]

    # WHA [H, 2H]: [enc1 h-part | pri1]; WHB [H, 3H]: [whh_rz | whh_n]
    wha = np.concatenate([W("enc_w1")[:, H:].T, W("pri_w1").T], axis=1)
    whb = np.ascontiguousarray(W("gru_whh").T)
    # B2 [1, 2H]: [pri_b1 | bhh_n]  (injected via ones-row matmul when nonzero)
    b2 = np.concatenate([W("pri_b1"), W("gru_bhh")[2 * H :]])[None, :]
    ew2t = np.ascontiguousarray(W("enc_w2").T)  # [H, Z]
    pw2t = np.ascontiguousarray(W("pri_w2").T)  # [H, Z]
    pzt = np.ascontiguousarray(W("phi_z_w").T)  # [Z, H]
    wihbt = np.ascontiguousarray(W("gru_wih")[:, H:].T)  # [H, 3H]
    decwt = np.ascontiguousarray(W("dec_w").T)  # [2H, H] rows [pz|h]
    actwt = np.ascontiguousarray(W("act_w").T)  # [H, A]
    # broadcast biases: [enc_b2 | pri_b2 | phi_z_b | dec_b | act_b]
    cbias = np.concatenate(
        [W("enc_b2"), W("pri_b2"), W("phi_z_b"), W("dec_b"), W("act_b")]
    )[None, :]  # [1, 2624]

    use_bias2 = bool(np.any(b2))
    use_cbias = bool(np.any(cbias))

    ident = np.eye(32, dtype=f32)

    shared = {
        "m12e": m12e.astype(BF16),
        "wha": wha.astype(BF16),
        "whb": whb.astype(BF16),
        "b2": b2.astype(BF16),
        "ew2t": ew2t.astype(BF16),
        "pw2t": pw2t.astype(BF16),
        "pzt": pzt.astype(BF16),
        "wihbt": wihbt.astype(BF16),
        "decwt": decwt.astype(BF16),
        "actwt": actwt.astype(BF16),
        "cbias": cbias.astype(BF16),
        "ident": ident.astype(BF16),
        "ident32": ident.astype(f32),
    }

    per_core = []
    for c in range(NCORES):
        lo, hi = c * BL, (c + 1) * BL
        xs = x[lo:hi]  # [BL, S]
        oh = np.zeros((S, A + 1, BL), f32)
        oh[:, A, :] = 1.0
        tt = np.arange(S)
        for b in range(BL):
            oh[tt, xs[b], b] = 1.0
        m = {
            "oh": oh.astype(BF16),
            "eps_s": np.ascontiguousarray(eps[:S, lo:hi, :]),  # [S, BL, Z]
            "eps_f": np.ascontiguousarray(eps[S, lo:hi, :]),  # [BL, Z]
            "h0bm": np.ascontiguousarray(h0[lo:hi]),  # [BL, H]
            "h0fm": np.ascontiguousarray(h0[lo:hi].T).astype(BF16),  # [H, BL]
        }
        m.update(shared)
        per_core.append(m)
    return per_core, (use_bias2, use_cbias)


def _build(s_steps=S, unroll=UNROLL, use_bias2=False, use_cbias=False):
    import concourse.bass as bass
    import concourse.tile as tile
    from concourse import bacc, mybir
    from concourse.bass import ds, ts

    f32 = mybir.dt.float32
    bf16 = mybir.dt.bfloat16
    i32 = mybir.dt.int32
    AF = mybir.ActivationFunctionType
    OP = mybir.AluOpType

    LN2 = 0.6931471805599453
    # deg-5 minimax poly of ln(m) on [1,2); C0 absorbs the magic-number
    # exponent bias: E_float = bitcast((bits>>23) | 0x4B000000) = 2^23+E+127
    LC = [-1.9367597429421461, 3.514087297000247, -2.440029762614449,
          1.1160900268323493, -0.2838268477821193, 0.03044900453867068]
    C0 = LC[0] - 127.0 * LN2

    nc = bacc.Bacc("TRN2", target_bir_lowering=False, debug=False,
                   enable_asserts=False, num_devices=NCORES)

    def dt_in(name, shape, dt):
        return nc.dram_tensor(name, list(shape), dt, kind="ExternalInput")

    d_oh = dt_in("oh", [s_steps, A + 1, BL], bf16)
    d_eps = dt_in("eps_s", [s_steps, BL, Z], f32)
    d_epsf = dt_in("eps_f", [BL, Z], f32)
    d_h0bm = dt_in("h0bm", [BL, H], f32)
    d_h0fm = dt_in("h0fm", [H, BL], bf16)
    d_m12e = dt_in("m12e", [A + 1, H + H3], bf16)
    d_wha = dt_in("wha", [H, 2 * H], bf16)
    d_whb = dt_in("whb", [H, H3], bf16)
    d_b2 = dt_in("b2", [1, 2 * H], bf16)
    d_ew2t = dt_in("ew2t", [H, Z], bf16)
    d_pw2t = dt_in("pw2t", [H, Z], bf16)
    d_pzt = dt_in("pzt", [Z, H], bf16)
    d_wihbt = dt_in("wihbt", [H, H3], bf16)
    d_decwt = dt_in("decwt", [2 * H, H], bf16)
    d_actwt = dt_in("actwt", [H, A], bf16)
    d_cbias = dt_in("cbias", [1, 2624], bf16)
    d_ident = dt_in("ident", [32, 32], bf16)
    d_ident32 = dt_in("ident32", [32, 32], f32)

    d_pred = nc.dram_tensor("pred", [BL, A], f32, kind="ExternalOutput")
    d_kld = nc.dram_tensor("kld", [1, 1], f32, kind="ExternalOutput")

    KT = H // 128  # 8 k-tiles of 128

    with tile.TileContext(nc) as tc, ExitStack() as ctx:
        wp = ctx.enter_context(tc.tile_pool(name="wp", bufs=1))
        app = ctx.enter_context(tc.tile_pool(name="app", bufs=1))
        sc = ctx.enter_context(tc.tile_pool(name="sc", bufs=1))
        sk = ctx.enter_context(tc.tile_pool(name="sk", bufs=1))
        sg = ctx.enter_context(tc.tile_pool(name="sg", bufs=2))
        skb = ctx.enter_context(tc.tile_pool(name="skb", bufs=1))
        st = ctx.enter_context(tc.tile_pool(name="st", bufs=3))
        dws = ctx.enter_context(tc.tile_pool(name="dws", bufs=2))
        pA = ctx.enter_context(tc.tile_pool(name="pA", bufs=1, space="PSUM"))
        pB = ctx.enter_context(tc.tile_pool(name="pB", bufs=1, space="PSUM"))
        pR = ctx.enter_context(tc.tile_pool(name="pR", bufs=1, space="PSUM"))

        # ---------------- resident weights ----------------
        def ld(name_tag, dram, shape, kt_rearr=False, dt=bf16):
            t = wp.tile(shape, dt, tag=name_tag)
            src = dram[:, :]
            if kt_rearr:
                src = src.rearrange("(k p) n -> p k n", p=128)
            nc.sync.dma_start(out=t, in_=src)
            return t

        w_m12e = ld("m12e", d_m12e, [A + 1, H + H3])
        w_wha = ld("wha", d_wha, [128, KT, 2 * H], True)
        w_whb = ld("whb", d_whb, [128, KT, H3], True)
        w_ew2t = ld("ew2t", d_ew2t, [128, KT, Z], True)
        w_pw2t = ld("pw2t", d_pw2t, [128, KT, Z], True)
        w_pzt = ld("pzt", d_pzt, [128, 2, H], True)
        w_wihbt = ld("wihbt", d_wihbt, [128, KT, H3], True)
        w_actwt = ld("actwt", d_actwt, [128, KT, A], True)
        w_id = ld("ident", d_ident, [32, 32])
        w_id32 = ld("ident32", d_ident32, [32, 32], dt=f32)
        if use_bias2:
            w_b2 = ld("b2", d_b2, [1, 2 * H])
            w_ones = wp.tile([1, BL], bf16, tag="ones")
            nc.vector.memset(w_ones, 1.0)
        if use_cbias:
            w_cb = wp.tile([BL, 2624], bf16, tag="cbias")
            nc.sync.dma_start(
                out=w_cb,
                in_=bass.AP(tensor=d_cbias, offset=0, ap=[[0, BL], [1, 2624]]),
            )
        w_ones32 = wp.tile([BL, 1], f32, tag="ones32")
        nc.vector.memset(w_ones32, 1.0)

        # ---------------- persistent state ----------------
        h_bm = app.tile([BL, H], f32, tag="h_bm")
        nc.sync.dma_start(out=h_bm, in_=d_h0bm[:, :])
        h_fm = app.tile([128, KT, BL], bf16, tag="h_fm")
        nc.sync.dma_start(out=h_fm, in_=d_h0fm[:, :].rearrange("(k p) b -> p k b", p=128))
        kacc = app.tile([BL, 1], f32, tag="kacc")
        nc.vector.memset(kacc, 0.0)
        eh_fm = app.tile([128, KT, BL], bf16, tag="eh_fm")
        ph_fm = app.tile([128, KT, BL], bf16, tag="ph_fm")
        pz_fm = app.tile([128, KT, BL], bf16, tag="pz_fm")
        z_fm = app.tile([128, 2, BL], bf16, tag="z_fm")
        dec_fm = app.tile([128, KT, BL], bf16, tag="dec_fm")

        def transpose_to_fm(src_bm, fm_tile, nk, pool, tagbase, dt=bf16):
            """src_bm [BL, nk*128] -> fm_tile [128, nk, BL] via PE
            transposes packed 4-per-psum-bank, ScalarE drains (casts)."""
            ident = w_id if dt == bf16 else w_id32
            for g in range((nk + 3) // 4):
                j0 = g * 4
                jn = min(4, nk - j0)
                tr = pool.tile([128, 4 * 32], dt, tag=tagbase)
                for j in range(jn):
                    nc.tensor.transpose(
                        tr[:, ts(j, 32)],
                        src_bm[:, (j0 + j) * 128 : (j0 + j + 1) * 128],
                        ident,
                    )
                nc.scalar.copy(
                    out=fm_tile[:, j0 : j0 + jn, :],
                    in_=tr[:, 0 : jn * 32].rearrange("p (j c) -> p j c", c=32),
                )

        def bitlog(y, out, width):
            """out = ln(y) elementwise on DVE, pure ALU. y is clobbered.
            Exponent: shift+debias in int, then DVE value-convert."""
            m_t = skb.tile([BL, width], f32, tag="bl_m%d" % width)
            nc.vector.tensor_scalar(out=m_t.bitcast(i32), in0=y.bitcast(i32),
                                    scalar1=0x007FFFFF, scalar2=0x3F800000,
                                    op0=OP.bitwise_and, op1=OP.bitwise_or)
            # raw exponent in int, value-convert to f32; -127 bias lives in C0
            nc.vector.tensor_scalar(out=y.bitcast(i32), in0=y.bitcast(i32),
                                    scalar1=23, scalar2=None,
                                    op0=OP.logical_shift_right)
            e_t = skb.tile([BL, width], f32, tag="bl_e%d" % width)
            nc.vector.tensor_copy(out=e_t, in_=y.bitcast(i32))
            m2 = skb.tile([BL, width], f32, tag="bl_m2%d" % width)
            nc.scalar.activation(out=m2, in_=m_t, func=AF.Square)
            m4 = skb.tile([BL, width], f32, tag="bl_m4%d" % width)
            nc.scalar.activation(out=m4, in_=m2, func=AF.Square)
            t1 = skb.tile([BL, width], f32, tag="bl_t1%d" % width)
            nc.vector.tensor_scalar(out=t1, in0=m_t, scalar1=LC[1], scalar2=C0,
                                    op0=OP.mult, op1=OP.add)
            t2 = skb.tile([BL, width], f32, tag="bl_t2%d" % width)
            nc.vector.tensor_scalar(out=t2, in0=m_t, scalar1=LC[3], scalar2=LC[2],
                                    op0=OP.mult, op1=OP.add)
            nc.vector.tensor_scalar(out=m_t, in0=m_t, scalar1=LC[5], scalar2=LC[4],
                                    op0=OP.mult, op1=OP.add)   # a3
            nc.vector.tensor_mul(t2, m2, t2)
            nc.vector.tensor_add(t1, t1, t2)
            nc.vector.tensor_mul(m4, m4, m_t)
            nc.vector.tensor_add(t1, t1, m4)
            nc.vector.scalar_tensor_tensor(out=out, in0=e_t, scalar=LN2, in1=t1,
                                           op0=OP.mult, op1=OP.add)
            return out

        def sp_chain(th, width, out_tag):
            """softplus from tanh(v/2) tile: y=1+e^v=2/(1-th); ln via bitlog."""
            nc.vector.tensor_scalar(out=th, in0=th, scalar1=-0.5, scalar2=0.5,
                                    op0=OP.mult, op1=OP.add)
            nc.vector.reciprocal(th, th)
            sp = sk.tile([BL, width], f32, tag=out_tag)
            bitlog(th, sp, width)
            return sp

        def kld_z_head(mean_psum, with_kld, eps_tile):
            """Fast path to z: sa-chain only. Returns (z_bm, deferred) where
            deferred() emits the sb/lambda/kel DVE work (call after the
            z->pz matmuls are emitted so it overlaps PE)."""
            cb_a = (w_cb[:, 0:Z] if with_kld else w_cb[:, Z : 2 * Z]) if use_cbias else None
            th_a = skb.tile([BL, Z], f32, tag="bl_tha")
            if cb_a is not None:
                raw = sk.tile([BL, Z], f32, tag="spraw")
                nc.vector.tensor_add(raw, mean_psum[:, 0:Z], cb_a)
                nc.scalar.activation(out=th_a, in_=raw, func=AF.Tanh, scale=0.5)
            else:
                nc.scalar.activation(out=th_a, in_=mean_psum[:, 0:Z], func=AF.Tanh,
                                     scale=0.5)
            if with_kld:
                # early PSUM reads so the bank frees before the deferred work
                th_b = skb.tile([BL, Z], f32, tag="bl_thb")
                if use_cbias:
                    rawb = sk.tile([BL, Z], f32, tag="sprawb")
                    nc.vector.tensor_add(rawb, mean_psum[:, Z : 2 * Z], w_cb[:, Z : 2 * Z])
                    nc.scalar.activation(out=th_b, in_=rawb, func=AF.Tanh, scale=0.5)
                else:
                    nc.scalar.activation(out=th_b, in_=mean_psum[:, Z : 2 * Z],
                                         func=AF.Tanh, scale=0.5)
                b_sb = sk.tile([BL, Z], f32, tag="b_sb")
                nc.scalar.copy(out=b_sb, in_=mean_psum[:, Z : 2 * Z])
                dd = sk.tile([BL, Z], f32, tag="dd")
                nc.vector.tensor_sub(dd, mean_psum[:, 0:Z], b_sb)
            sa = sp_chain(th_a, Z, "sa")
            zt = sk.tile([BL, Z], f32, tag="zt")
            nc.vector.tensor_mul(zt, eps_tile, sa)
            z_bm = sk.tile([BL, Z], bf16, tag="z_bm")
            nc.vector.tensor_add(z_bm, zt, mean_psum[:, 0:Z])
            if not with_kld:
                return z_bm, (lambda: None)

            def deferred():
                sb = sp_chain(th_b, Z, "sb")
                rho = skb.tile([BL, Z], f32, tag="bl_tha")
                nc.vector.reciprocal(rho, sa)
                nc.vector.tensor_mul(rho, sb, rho)
                lam = sk.tile([BL, Z], f32, tag="lam")
                bitlog(rho, lam, Z)
                sa2 = skb.tile([BL, Z], f32, tag="bl_m2%d" % Z)
                nc.scalar.activation(out=sa2, in_=sa, func=AF.Square)
                sb2 = skb.tile([BL, Z], f32, tag="bl_m4%d" % Z)
                nc.scalar.activation(out=sb2, in_=sb, func=AF.Square)
                dd2 = skb.tile([BL, Z], f32, tag="bl_t1%d" % Z)
                nc.scalar.activation(out=dd2, in_=dd, func=AF.Square)
                nc.vector.tensor_add(sa2, sa2, dd2)
                nc.vector.reciprocal(sb2, sb2)
                nc.vector.tensor_mul(sa2, sa2, sb2)
                red = sk.tile([BL, 1], f32, tag="red")
                nc.vector.scalar_tensor_tensor(
                    out=lam, in0=lam, scalar=2.0, in1=sa2,
                    op0=OP.mult, op1=OP.add, accum_out=red,
                )
                nc.vector.tensor_add(kacc, kacc, red)
            return z_bm, deferred

        def z_to_fm_pz(z_bm):
            tr = pA.tile([128, 4 * 32], bf16, tag="A")
            for j in range(2):
                nc.tensor.transpose(tr[:, ts(j, 32)], z_bm[:, ts(j, 128)], w_id)
            nc.scalar.copy(out=z_fm[:, 0:2, :],
                           in_=tr[:, 0:64].rearrange("p (j c) -> p j c", c=32))
            pzp = pB.tile([BL, H], f32, tag="B")
            for k in range(2):
                for n in range(2):
                    nc.tensor.matmul(pzp[:, ts(n, 512)], z_fm[:, k, :],
                                     w_pzt[:, k, ts(n, 512)],
                                     start=(k == 0), stop=(k == 1))
            pz_bm = sc.tile([BL, H], bf16, tag="pz_bm")
            if use_cbias:
                pzb = sc.tile([BL, H], f32, tag="pzb")
                nc.vector.tensor_add(pzb, pzp, w_cb[:, 2 * Z : 2 * Z + H])
                nc.scalar.activation(out=pz_bm, in_=pzb, func=AF.Relu)
            else:
                nc.scalar.activation(out=pz_bm, in_=pzp, func=AF.Relu)
            transpose_to_fm(pz_bm, pz_fm, KT, pB, "B")

        def pri1_matmuls(target):
            first = True
            if use_bias2:
                for n in range(2):
                    nc.tensor.matmul(target[:, ts(n, 512)], w_ones,
                                     w_b2[:, ts(n, 512)], start=True, stop=False)
                first = False
            for k in range(KT):
                for n in range(2):
                    nc.tensor.matmul(target[:, ts(n, 512)], h_fm[:, k, :],
                                     w_wha[:, k, 1024 + n * 512 : 1024 + (n + 1) * 512],
                                     start=(first and k == 0), stop=(k == KT - 1))

        def step_body(iv):
            # ---- streams ----
            oh_t = st.tile([A + 1, BL], bf16, tag="oh")
            nc.sync.dma_start(out=oh_t,
                              in_=d_oh[ds(iv, 1), :, :].rearrange("o p b -> (o p) b"))
            eps_t = st.tile([BL, Z], f32, tag="eps")
            nc.sync.dma_start(out=eps_t,
                              in_=d_eps[ds(iv, 1), :, :].rearrange("o p b -> (o p) b"))

            # ---- enc1 / pri1 ----
            enc1 = pA.tile([BL, H], f32, tag="A")
            pri1 = pB.tile([BL, H], f32, tag="B")
            for n in range(2):
                nc.tensor.matmul(enc1[:, ts(n, 512)], oh_t, w_m12e[:, ts(n, 512)],
                                 start=True, stop=False)
            for k in range(KT):
                for n in range(2):
                    nc.tensor.matmul(enc1[:, ts(n, 512)], h_fm[:, k, :],
                                     w_wha[:, k, ts(n, 512)],
                                     start=False, stop=(k == KT - 1))
            pri1_matmuls(pri1)

            eh_bm = sc.tile([BL, H], bf16, tag="eh_bm")
            nc.scalar.activation(out=eh_bm, in_=enc1, func=AF.Relu)
            ph_bm = sc.tile([BL, H], bf16, tag="ph_bm")
            nc.scalar.activation(out=ph_bm, in_=pri1, func=AF.Relu)

            transpose_to_fm(eh_bm, eh_fm, KT, pA, "A")
            transpose_to_fm(ph_bm, ph_fm, KT, pB, "B")

            # ---- enc2 / pri2 (one bank: cols 0:256 enc2, 256:512 pri2) ----
            e2p2 = pB.tile([BL, 512], f32, tag="B")
            for k in range(KT):
                nc.tensor.matmul(e2p2[:, 0:Z], eh_fm[:, k, :], w_ew2t[:, k, :],
                                 start=(k == 0), stop=(k == KT - 1))
            for k in range(KT):
                nc.tensor.matmul(e2p2[:, Z : 2 * Z], ph_fm[:, k, :], w_pw2t[:, k, :],
                                 start=(k == 0), stop=(k == KT - 1))

            # ---- rz: oh + h parts early (overlaps KLD on DVE/ACT) ----
            rz = pR.tile([BL, 2 * H], f32, tag="R")
            for n in range(4):
                nc.tensor.matmul(rz[:, ts(n, 512)], oh_t,
                                 w_m12e[:, 1024 + n * 512 : 1024 + (n + 1) * 512],
                                 start=True, stop=False)
            for k in range(KT):
                for n in range(4):
                    nc.tensor.matmul(rz[:, ts(n, 512)], h_fm[:, k, :],
                                     w_whb[:, k, ts(n, 512)],
                                     start=False, stop=False)

            # ---- KLD + z (sa fast path; sb/lambda deferred past PE work) ----
            z_bm, kld_deferred = kld_z_head(e2p2, True, eps_t)

            # ---- z -> fm -> phi_z -> pz (+transposes) ----
            z_to_fm_pz(z_bm)
            kld_deferred()

            # ---- rz pz-part, inn, hn ----
            for k in range(KT):
                for n in range(4):
                    nc.tensor.matmul(rz[:, ts(n, 512)], pz_fm[:, k, :],
                                     w_wihbt[:, k, ts(n, 512)],
                                     start=False, stop=(k == KT - 1))
            inn = pA.tile([BL, H], f32, tag="A")
            for n in range(2):
                nc.tensor.matmul(inn[:, ts(n, 512)], oh_t,
                                 w_m12e[:, 3072 + n * 512 : 3072 + (n + 1) * 512],
                                 start=True, stop=False)
            for k in range(KT):
                for n in range(2):
                    nc.tensor.matmul(inn[:, ts(n, 512)], pz_fm[:, k, :],
                                     w_wihbt[:, k, 2048 + n * 512 : 2048 + (n + 1) * 512],
                                     start=False, stop=(k == KT - 1))
            hn = pB.tile([BL, H], f32, tag="B")
            first = True
            if use_bias2:
                for n in range(2):
                    nc.tensor.matmul(hn[:, ts(n, 512)], w_ones,
                                     w_b2[:, 1024 + n * 512 : 1024 + (n + 1) * 512],
                                     start=True, stop=False)
                first = False
            for k in range(KT):
                for n in range(2):
                    nc.tensor.matmul(hn[:, ts(n, 512)], h_fm[:, k, :],
                                     w_whb[:, k, 2048 + n * 512 : 2048 + (n + 1) * 512],
                                     start=(first and k == 0), stop=(k == KT - 1))

            # ---- gates (2 chunks of 512; sigmoid == (1+tanh(x/2))/2) ----
            for cch in range(2):
                cs = ts(cch, 512)
                th_r = sg.tile([BL, 512], f32, tag="th_r")
                nc.scalar.activation(out=th_r, in_=rz[:, cch * 512 : (cch + 1) * 512],
                                     func=AF.Tanh, scale=0.5)
                th_z = sg.tile([BL, 512], f32, tag="th_z")
                nc.scalar.activation(
                    out=th_z, in_=rz[:, 1024 + cch * 512 : 1024 + (cch + 1) * 512],
                    func=AF.Tanh, scale=0.5)
                # r*hn = 0.5*(th_r*hn + hn)
                nc.vector.tensor_mul(th_r, th_r, hn[:, cs])
                nc.vector.tensor_add(th_r, th_r, hn[:, cs])
                nc.vector.scalar_tensor_tensor(out=th_r, in0=th_r, scalar=0.5,
                                               in1=inn[:, cs], op0=OP.mult, op1=OP.add)
                nc.scalar.activation(out=th_r, in_=th_r, func=AF.Tanh)  # n
                d1 = sg.tile([BL, 512], f32, tag="d1")
                nc.vector.tensor_sub(d1, h_bm[:, cs], th_r)             # h-n
                # zg*(h-n) = 0.5*(th_z*d1 + d1); h_new = that + n
                nc.vector.tensor_mul(th_z, th_z, d1)
                nc.vector.tensor_add(th_z, th_z, d1)
                nc.vector.scalar_tensor_tensor(out=h_bm[:, cs], in0=th_z, scalar=0.5,
                                               in1=th_r, op0=OP.mult, op1=OP.add)
                # transpose this chunk's h_new immediately (PE work now)
                tr = pA.tile([128, 4 * 32], f32, tag="A")
                for j in range(4):
                    nc.tensor.transpose(
                        tr[:, ts(j, 32)],
                        h_bm[:, cch * 512 + j * 128 : cch * 512 + (j + 1) * 128],
                        w_id32)
                nc.scalar.copy(
                    out=h_fm[:, cch * 4 : cch * 4 + 4, :],
                    in_=tr[:, 0:128].rearrange("p (j c) -> p j c", c=32))

        if s_steps >= unroll and s_steps % unroll == 0:
            with tc.For_i(0, s_steps, unroll,
                          hint_engines=(mybir.EngineType.PE,)) as iv:
                for uu in range(unroll):
                    step_body(iv if uu == 0 else iv + uu)
        else:
            for t in range(s_steps):
                step_body(t)

        # ================= final phase =================
        pri1 = pB.tile([BL, H], f32, tag="B")
        pri1_matmuls(pri1)
        ph_bm = sc.tile([BL, H], bf16, tag="ph_bm")
        nc.scalar.activation(out=ph_bm, in_=pri1, func=AF.Relu)
        transpose_to_fm(ph_bm, ph_fm, KT, pB, "B")
        p2 = pB.tile([BL, 512], f32, tag="B")
        for k in range(KT):
            nc.tensor.matmul(p2[:, 0:Z], ph_fm[:, k, :], w_pw2t[:, k, :],
                             start=(k == 0), stop=(k == KT - 1))
        epsf_t = st.tile([BL, Z], f32, tag="eps")
        nc.sync.dma_start(out=epsf_t, in_=d_epsf[:, :])
        z_bm, _ = kld_z_head(p2, False, epsf_t)
        z_to_fm_pz(z_bm)

        # dec = relu(cat([pz, h]) @ dec_w.T + dec_b): stream decwt k-tiles
        decp = pR.tile([BL, H], f32, tag="R")
        for k in range(2 * KT):
            stat = pz_fm[:, k, :] if k < KT else h_fm[:, k - KT, :]
            for n in range(2):
                dwt = dws.tile([128, 512], bf16, tag="dw")
                nc.sync.dma_start(out=dwt,
                                  in_=d_decwt[k * 128 : (k + 1) * 128, ts(n, 512)])
                nc.tensor.matmul(decp[:, ts(n, 512)], stat, dwt,
                                 start=(k == 0), stop=(k == 2 * KT - 1))
        dec_bm = sc.tile([BL, H], bf16, tag="pz_bm")
        if use_cbias:
            decb = sc.tile([BL, H], f32, tag="pzb")
            nc.vector.tensor_add(decb, decp, w_cb[:, 2 * Z + H : 2 * Z + 2 * H])
            nc.scalar.activation(out=dec_bm, in_=decb, func=AF.Relu)
        else:
            nc.scalar.activation(out=dec_bm, in_=decp, func=AF.Relu)
        transpose_to_fm(dec_bm, dec_fm, KT, pB, "B")
        actp = pA.tile([BL, A], f32, tag="A")
        for k in range(KT):
            nc.tensor.matmul(actp, dec_fm[:, k, :], w_actwt[:, k, :],
                             start=(k == 0), stop=(k == KT - 1))
        pred_sb = sk.tile([BL, A], f32, tag="pred")
        if use_cbias:
            nc.vector.tensor_add(pred_sb, actp,
                                 w_cb[:, 2 * Z + 2 * H : 2 * Z + 2 * H + A])
        else:
            nc.scalar.copy(out=pred_sb, in_=actp)
        nc.sync.dma_start(out=d_pred[:, :], in_=pred_sb)

        # KLD partition-sum via ones matmul: [1,1] = kacc.T @ ones
        ksum = pB.tile([1, 1], f32, tag="B")
        nc.tensor.matmul(ksum, kacc, w_ones32, start=True, stop=True)
        ksb = sk.tile([1, 1], f32, tag="ksb")
        nc.scalar.copy(out=ksb, in_=ksum)
        nc.sync.dma_start(out=d_kld[:, :], in_=ksb)

    nc.compile()
    return nc


_NC_CACHE = {}


def _get_nc(s_steps=S, unroll=UNROLL, use_bias2=False, use_cbias=False):
    key = (s_steps, unroll, use_bias2, use_cbias)
    if key not in _NC_CACHE:
        _NC_CACHE[key] = _build(s_steps, unroll, use_bias2, use_cbias)
    return _NC_CACHE[key]


def kernel(**inputs):
    from concourse.bass_utils import run_bass_kernel_spmd

    per_core, (ub2, ucb) = _host_prep(inputs)
    nc = _get_nc(use_bias2=ub2, use_cbias=ucb)
    res = run_bass_kernel_spmd(nc, per_core, core_ids=list(range(NCORES)))
    preds = []
    kld_raw = 0.0
    for r in res.results:
        preds.append(np.asarray(r["pred"], np.float32))
        kld_raw += float(np.asarray(r["kld"]).reshape(-1)[0])
    pred = np.concatenate(preds, axis=0)
    kld = np.float32(0.5 * (kld_raw - float(B) * S * Z))
    return pred, kld
